# revision 2
# baseline (speedup 1.0000x reference)
"""Causal self-attention (B=4, T=2048, C=1024, H=16) on 8 TRN2 NeuronCores.

Sharding: tensor-parallel over batch x head-halves. Core c handles batch
b = c//2 and heads 8*(c%2) .. 8*(c%2)+8. Each core computes its 8 heads'
QKV projection, causal attention, and a partial c_proj contribution
(contracting only its 512 c_in rows). The host sums the partial outputs
per batch and adds the biases. The V bias is folded into the host-side
output bias (softmax rows sum to 1, so Y_norm@(V+1*bv^T)@W2 =
Y_norm@V@W2 + bv^T@W2), so the device never touches it.

Per-core kernel layout (all matmuls bf16 with fp32 PSUM accumulation):
- xT uploaded column-blocked (per 512-wide t-block) and wqk split so the
  pair-0 quads land first: V tiles and pair-0 Q/K stream behind the DMA
  and attention starts ~15us in, keeping the PE HAM-warm from the start.
- S^T = K Q^T per (head-pair, q-block, k-tile); two heads run
  concurrently via PE row-tiling (K=64 at partitions 0 and 64). On
  diagonal k-tiles S, exp, and att@V are all restricted to the causally
  reachable column range [128m, 512); dead columns are never computed or
  read. Only the 128-wide diagonal band needs an elementwise {0,1} mask
  multiply (GpSimd).
- exp on ACT with fused 1/8 scale.
- att@V with [V_h | 1] stationary gives Y^T[d, q] plus row-sum r[q] in
  one accumulation. Normalization: 1-lane DVE reciprocal of the row-sum
  row straight out of PSUM, DRAM round trip for the partition broadcast,
  one DVE mul into per-(pair, q-block) ynt tiles.
- c_proj consumes ynt tiles as lhsT; the two head-halves write separate
  bf16 outputs (host sums), so c_proj streams during attention and the
  tail is just the last q-block. Evictions ride ScalarE where ACT is
  otherwise idle (V ramp, qk0 ramp, c_proj tail) and DVE elsewhere.
"""

import os
import numpy as np

# Tile's fine-grained (subtile) dependency tracker misses a required
# semaphore for some partition-sliced read / column-sliced write combos in
# this kernel (nondeterministic corruption on HW, sim-clean). Coarse
# tile-level deps are correct. Must be set before the first tile() call.
os.environ["BY_DEFAULT_DISABLE_SUBTILE_DEPS"] = "1"

B, T, C, H = 4, 2048, 1024, 16
HL = 8          # heads per core
D = 64          # head dim
NCORES = 8
NTB = 4         # t-blocks of 512 over T
NCHUNK = 8      # c_in chunks of 128 over C
NTT = 16        # t-tiles of 128 over T
VGW = 65        # V group width per head (64 cols + ones col)


def _build_nc():
    import concourse.bass as bass
    import concourse.tile as tile
    from concourse import bacc, mybir

    F32 = mybir.dt.float32
    BF16 = mybir.dt.bfloat16
    EXP = mybir.ActivationFunctionType.Exp
    CPY = mybir.ActivationFunctionType.Copy
    IDN = mybir.ActivationFunctionType.Identity

    nc = bacc.Bacc("TRN2", target_bir_lowering=False, debug=False,
                   num_devices=NCORES)

    d_xT = nc.dram_tensor("xT", (C, T), BF16, kind="ExternalInput").ap()
    d_wqk = nc.dram_tensor("wqk", (C, 1024), BF16, kind="ExternalInput").ap()
    d_bqk = nc.dram_tensor("bqk", (128, 8), F32, kind="ExternalInput").ap()
    d_wv = nc.dram_tensor("wv", (C, 512), BF16, kind="ExternalInput").ap()
    d_w2 = nc.dram_tensor("w2", (512, C), BF16, kind="ExternalInput").ap()
    d_mask = nc.dram_tensor("mask", (128, 256), BF16,
                            kind="ExternalInput").ap()
    d_ones = nc.dram_tensor("onesr", (128, 8), BF16,
                            kind="ExternalInput").ap()
    d_out1 = nc.dram_tensor("out1", (T, C), BF16, kind="ExternalOutput").ap()
    d_out2 = nc.dram_tensor("out2", (T, C), BF16, kind="ExternalOutput").ap()

    with tile.TileContext(nc) as tc:
        with tc.tile_pool(name="persist", bufs=1) as persist:
            xts = [[persist.tile([128, 512], BF16, name=f"xt{c}_{tb}",
                                 tag=f"xt{c}_{tb}") for tb in range(NTB)]
                   for c in range(NCHUNK)]
            wv_sb = [persist.tile([128, 512], BF16, name=f"wv{c}",
                                  tag=f"wv{c}") for c in range(NCHUNK)]
            wqkA = [persist.tile([128, 256], BF16, name=f"wqa{c}",
                                 tag=f"wqa{c}") for c in range(NCHUNK)]
            wqkB = [persist.tile([128, 768], BF16, name=f"wqb{c}",
                                 tag=f"wqb{c}") for c in range(NCHUNK)]
            w2_sb = [persist.tile([128, C], BF16, name=f"w2{g}", tag=f"w2{g}")
                     for g in range(4)]
            bq_sb = persist.tile([128, 8], F32, name="bq", tag="bq")
            mk = persist.tile([128, 256], BF16, name="mk", tag="mk")
            vt = [persist.tile([128, HL * VGW], BF16, name=f"v{t}",
                               tag=f"v{t}") for t in range(NTT)]
            qt = [[persist.tile([128, 512], BF16, name=f"qt{g}_{tb}",
                                tag=f"qt{g}_{tb}") for tb in range(NTB)]
                  for g in range(4)]
            kt_ = [[persist.tile([128, 512], BF16, name=f"kt{g}_{tb}",
                                 tag=f"kt{g}_{tb}") for tb in range(NTB)]
                   for g in range(4)]
            ynt = [[persist.tile([128, 512], BF16, name=f"yn{g}_{qb}",
                                 tag=f"yn{g}_{qb}") for qb in range(NTB)]
                   for g in range(4)]

            # ---- input DMAs, ordered so attention can start ~15us in ----
            nc.sync.dma_start(bq_sb[:], d_bqk[:])
            nc.sync.dma_start(mk[:], d_mask[:])
            for t in range(NTT):
                vg0 = vt[t][:].rearrange("p (g c) -> p g c", g=HL)
                nc.sync.dma_start(
                    vg0[:, :, 64:65],
                    d_ones[:, 0:HL].rearrange("p (g o) -> p g o", o=1))
            # wv + xT t-block 0, chunk-interleaved
            for c in range(NCHUNK):
                nc.sync.dma_start(wv_sb[c][:], d_wv[c * 128:(c + 1) * 128, :])
                nc.sync.dma_start(xts[c][0][:],
                                  d_xT[c * 128:(c + 1) * 128, 0:512])
            # wqk part A: the pair-0 quads (q cols 0:128, k cols 512:640)
            for c in range(NCHUNK):
                src = d_wqk[c * 128:(c + 1) * 128, :].rearrange(
                    "p (s w) -> p s w", s=2)
                dst = wqkA[c][:].rearrange("p (s w) -> p s w", s=2)
                nc.sync.dma_start(dst, src[:, :, 0:128])
            # xT t-blocks 1..3
            for tb in range(1, NTB):
                for c in range(NCHUNK):
                    nc.sync.dma_start(
                        xts[c][tb][:],
                        d_xT[c * 128:(c + 1) * 128, tb * 512:(tb + 1) * 512])
            # wqk part B: pairs 1-3 (q cols 128:512, k cols 640:1024)
            for c in range(NCHUNK):
                src = d_wqk[c * 128:(c + 1) * 128, :].rearrange(
                    "p (s w) -> p s w", s=2)
                dst = wqkB[c][:].rearrange("p (s w) -> p s w", s=2)
                nc.sync.dma_start(dst, src[:, :, 128:512])
            for g in range(4):
                nc.sync.dma_start(w2_sb[g][:], d_w2[g * 128:(g + 1) * 128, :])

            def wqk_ap(c, mt):
                if mt == 0:
                    return wqkA[c][:, 0:128]
                if mt == 4:
                    return wqkA[c][:, 128:256]
                if mt < 4:
                    return wqkB[c][:, (mt - 1) * 128:mt * 128]
                return wqkB[c][:, 384 + (mt - 5) * 128:384 + (mt - 4) * 128]

            with tc.tile_pool(name="pt", bufs=6) as pt_pool, \
                 tc.tile_pool(name="yu", bufs=4) as yu_pool, \
                 tc.tile_pool(name="rr", bufs=4) as rr_pool, \
                 tc.tile_pool(name="bc", bufs=4) as bc_pool, \
                 tc.tile_pool(name="rdram", bufs=4, space="DRAM") as rd_pool, \
                 tc.tile_pool(name="outp", bufs=4) as ob_pool, \
                 tc.tile_pool(name="psp", bufs=2, space="PSUM") as ps_p, \
                 tc.tile_pool(name="pss", bufs=2, space="PSUM") as ps_s, \
                 tc.tile_pool(name="psy", bufs=2, space="PSUM") as ps_y:

                # ---- V production (streams behind the xT upload) ----
                with nc.named_scope("vproj"):
                    for tb in range(NTB):
                        for tt in range(4):
                            ps = ps_p.tile([128, 512], F32, name="psp",
                                           tag="psp")
                            for c in range(NCHUNK):
                                nc.tensor.matmul(
                                    ps[:], xts[c][tb][:, tt * 128:(tt + 1) * 128],
                                    wv_sb[c][:],
                                    start=(c == 0), stop=(c == NCHUNK - 1))
                            vg = vt[4 * tb + tt][:].rearrange(
                                "p (g c) -> p g c", g=HL)
                            # ACT is idle during the ramp; evict there
                            nc.scalar.activation(
                                vg[:, :, 0:64],
                                ps[:].rearrange("p (g c) -> p g c", g=HL),
                                CPY)

                def qk_pair(hp):
                    with nc.named_scope(f"qk{hp}"):
                        for tb in range(NTB):
                            for mt in (hp, hp + 4):
                                dst = (qt if mt < 4 else kt_)[hp][tb]
                                ps = ps_p.tile([128, 512], F32, name="psp",
                                               tag="psp")
                                for c in range(NCHUNK):
                                    nc.tensor.matmul(
                                        ps[:], wqk_ap(c, mt), xts[c][tb][:],
                                        start=(c == 0),
                                        stop=(c == NCHUNK - 1))
                                if hp == 0:
                                    nc.scalar.activation(
                                        dst[:], ps[:], IDN,
                                        bias=bq_sb[:, mt:mt + 1])
                                else:
                                    nc.vector.tensor_scalar_add(
                                        dst[:], ps[:], bq_sb[:, mt:mt + 1])

                def attention_pair(hp):
                    with nc.named_scope(f"attn{hp}"):
                        for qb in range(NTB):
                            nkt = 4 * qb + 4
                            psY = {}
                            for side in (0, 1):
                                psY[side] = ps_y.tile([VGW, 512], F32,
                                                      name="psy", tag="psy")
                            for ktile in range(nkt):
                                m = ktile - 4 * qb
                                r0 = 128 * max(m, 0)
                                kb, kc = divmod(ktile, 4)
                                psS = ps_s.tile([128, 1024], F32,
                                                name="pss", tag="pss")
                                pt = pt_pool.tile([128, 1024], BF16,
                                                  name="pt", tag="pt")
                                # S^T: two heads row-tiled, concurrent
                                for side, po in ((0, 0), (1, 64)):
                                    nc.tensor.matmul(
                                        psS[:, side * 512 + r0:(side + 1) * 512],
                                        kt_[hp][kb][po:po + 64,
                                                    kc * 128:(kc + 1) * 128],
                                        qt[hp][qb][po:po + 64, r0:512])
                                if m <= 0:
                                    nc.scalar.activation(pt[:], psS[:], EXP,
                                                         scale=0.125)
                                else:
                                    ps2 = psS[:].rearrange("p (s w) -> p s w",
                                                           s=2)
                                    pt2 = pt[:].rearrange("p (s w) -> p s w",
                                                          s=2)
                                    nc.scalar.activation(pt2[:, :, r0:512],
                                                         ps2[:, :, r0:512],
                                                         EXP, scale=0.125)
                                if m >= 0:
                                    # mask only the 128-wide diagonal band
                                    pt2 = pt[:].rearrange("p (s w) -> p s w",
                                                          s=2)
                                    mkb = mk[:].rearrange("p (s w) -> p s w",
                                                          s=2)
                                    nc.gpsimd.tensor_mul(
                                        pt2[:, :, r0:r0 + 128],
                                        pt2[:, :, r0:r0 + 128], mkb)
                                for side, po in ((0, 0), (1, 64)):
                                    h = 2 * hp + side
                                    nc.tensor.matmul(
                                        psY[side][:, r0:512],
                                        vt[ktile][:, h * VGW:(h + 1) * VGW],
                                        pt[:, side * 512 + r0:(side + 1) * 512],
                                        start=(ktile == 0),
                                        stop=(ktile == nkt - 1))
                            # normalize: 1/rowsum straight from PSUM, DRAM
                            # round trip for the partition broadcast
                            for side, po in ((0, 0), (1, 64)):
                                rr = rr_pool.tile([1, 512], F32, name="rr",
                                                  tag="rr")
                                nc.vector.reciprocal(rr[:],
                                                     psY[side][64:65, :])
                                yu = yu_pool.tile([64, 512], F32, name="yu",
                                                  tag="yu")
                                nc.vector.tensor_copy(yu[:],
                                                      psY[side][0:64, :])
                                rd = rd_pool.tile([1, 512], F32, name="rd",
                                                  tag="rd")
                                nc.sync.dma_start(rd[:], rr[:])
                                bc = bc_pool.tile([64, 512], F32, name="bc",
                                                  tag="bc")
                                rd_ap = rd[:]
                                nc.sync.dma_start(
                                    bc[:],
                                    bass.AP(tensor=rd_ap.tensor,
                                            offset=rd_ap.offset,
                                            ap=[[0, 64]] + list(rd_ap.ap[1:])))
                                nc.vector.tensor_mul(
                                    ynt[hp][qb][po:po + 64, :], yu[:], bc[:])

                def cproj_half(gs, d_out, on_scalar):
                    with nc.named_scope(f"cproj{gs[0]}"):
                        for qb in range(NTB):
                            for tt in range(4):
                                t = 4 * qb + tt
                                for nb in range(2):
                                    nsl = slice(nb * 512, (nb + 1) * 512)
                                    ps = ps_p.tile([128, 512], F32, name="psp",
                                                   tag="psp")
                                    for j, g in enumerate(gs):
                                        nc.tensor.matmul(
                                            ps[:],
                                            ynt[g][qb][:, tt * 128:(tt + 1) * 128],
                                            w2_sb[g][:, nsl],
                                            start=(j == 0),
                                            stop=(j == len(gs) - 1))
                                    ob = ob_pool.tile([128, 512], BF16,
                                                      name="ob", tag="ob")
                                    if on_scalar:
                                        nc.scalar.activation(ob[:], ps[:],
                                                             CPY)
                                    else:
                                        nc.vector.tensor_copy(ob[:], ps[:])
                                    nc.sync.dma_start(
                                        d_out[t * 128:(t + 1) * 128, nsl],
                                        ob[:])

                # interleave: QK(hp+1) overlaps attention(hp); cproj of
                # heads 0-3 fills PE gaps during attention of pair 3;
                # cproj of heads 4-7 streams per q-block behind attn3.
                qk_pair(0)
                for hp in range(4):
                    if hp + 1 < 4:
                        qk_pair(hp + 1)
                    if hp == 3:
                        cproj_half((0, 1), d_out1, False)
                    attention_pair(hp)
                cproj_half((2, 3), d_out2, True)

    nc.compile()
    return nc


def _shard_inputs(x, c_attn_w, c_attn_b, c_proj_w):
    import ml_dtypes
    bf16 = ml_dtypes.bfloat16
    i = np.arange(128)[:, None]
    j = np.arange(128)[None, :]
    tri = (i <= j)
    mask = np.concatenate([tri, tri], axis=1).astype(bf16)
    ones8 = np.ones((128, 8), bf16)
    in_maps = []
    for core in range(NCORES):
        b, half = core // 2, core % 2
        h0 = half * HL
        lo, hi = h0 * D, (h0 + HL) * D
        wq = c_attn_w[:, lo:hi]
        wk = c_attn_w[:, C + lo:C + hi]
        wv = c_attn_w[:, 2 * C + lo:2 * C + hi]
        bqk = np.concatenate([c_attn_b[lo:hi], c_attn_b[C + lo:C + hi]])
        in_maps.append({
            "xT": np.ascontiguousarray(x[b].T).astype(bf16),
            "wqk": np.ascontiguousarray(
                np.concatenate([wq, wk], axis=1)).astype(bf16),
            "bqk": np.ascontiguousarray(
                bqk.reshape(8, 128).T, dtype=np.float32),
            "wv": np.ascontiguousarray(wv).astype(bf16),
            "w2": np.ascontiguousarray(c_proj_w[lo:hi, :]).astype(bf16),
            "mask": mask,
            "onesr": ones8,
        })
    return in_maps


def _run(x, c_attn_w, c_attn_b, c_proj_w, c_proj_b, trace=False):
    from concourse import bass_utils
    x = np.asarray(x, np.float32)
    c_attn_w = np.asarray(c_attn_w, np.float32)
    c_attn_b = np.asarray(c_attn_b, np.float32)
    c_proj_w = np.asarray(c_proj_w, np.float32)
    c_proj_b = np.asarray(c_proj_b, np.float32)

    nc = _build_nc()
    in_maps = _shard_inputs(x, c_attn_w, c_attn_b, c_proj_w)
    res = bass_utils.run_bass_kernel_spmd(nc, in_maps,
                                          core_ids=list(range(NCORES)),
                                          trace=trace)
    # softmax rows sum to 1 => the V bias contributes bv^T @ c_proj_w to
    # every (b, t) row; fold it into the host-side bias add.
    bias = c_proj_b + c_attn_b[2 * C:] @ c_proj_w
    y = np.empty((B, T, C), np.float32)
    for b in range(B):
        acc = None
        for core in (2 * b, 2 * b + 1):
            for name in ("out1", "out2"):
                part = np.asarray(res.results[core][name], np.float32)
                acc = part if acc is None else acc + part
        y[b] = acc + bias[None, :]
    return y, res


def kernel(x, c_attn_w, c_attn_b, c_proj_w, c_proj_b):
    y, _ = _run(x, c_attn_w, c_attn_b, c_proj_w, c_proj_b, trace=False)
    return y


# revision 5
# speedup vs baseline: 1.3828x; 1.3828x over previous
"""Causal self-attention (B=4, T=2048, C=1024, H=16) on 8 TRN2 NeuronCores.

Sharding: tensor-parallel over batch x head-halves. Core c handles batch
b = c//2 and heads 8*(c%2) .. 8*(c%2)+8. Each core computes its 8 heads'
QKV projection, causal attention, and a partial c_proj contribution
(contracting only its 512 c_in rows). The host sums the partial outputs
per batch and adds the biases. The V bias is folded into the host-side
output bias (softmax rows sum to 1, so Y_norm@(V+1*bv^T)@W2 =
Y_norm@V@W2 + bv^T@W2), so the device never touches it.

Per-core kernel layout (all matmuls bf16 with fp32 PSUM accumulation):
- xT uploaded column-blocked (per 512-wide t-block) and wqk split so the
  pair-0 quads land first: V tiles and pair-0 Q/K stream behind the DMA
  and attention starts ~15us in, keeping the PE HAM-warm from the start.
- S^T = K Q^T per (head-pair, q-block, k-tile); two heads run
  concurrently via PE row-tiling (K=64 at partitions 0 and 64). On
  diagonal k-tiles S, exp, and att@V are all restricted to the causally
  reachable column range [128m, 512); dead columns are never computed or
  read. Only the 128-wide diagonal band needs an elementwise {0,1} mask
  multiply (GpSimd).
- exp on ACT with fused 1/8 scale.
- att@V with [V_h | 1] stationary gives Y^T[d, q] plus row-sum r[q] in
  one accumulation. Unnormalized Y^T+r evicted to SBUF; the r row goes
  through a DMA reshape round-trip ([1,512] -> [128,4] for a wide DVE
  reciprocal -> back) and a DRAM partition-broadcast; normalization is
  one DVE mul into per-(pair, q-block) ynt tiles.
- c_proj consumes ynt tiles as lhsT; the two head-halves write separate
  bf16 outputs (host sums), so c_proj streams during attention and the
  tail is just the last q-block. Evictions ride ScalarE where ACT is
  otherwise idle (V ramp, qk0 ramp, c_proj tail) and DVE elsewhere.
"""

import os
import numpy as np

# Tile's fine-grained (subtile) dependency tracker misses a required
# semaphore for some partition-sliced read / column-sliced write combos in
# this kernel (nondeterministic corruption on HW, sim-clean). Coarse
# tile-level deps are correct. Must be set before the first tile() call.
os.environ["BY_DEFAULT_DISABLE_SUBTILE_DEPS"] = "1"

B, T, C, H = 4, 2048, 1024, 16
HL = 8          # heads per core
D = 64          # head dim
NCORES = 8
NTB = 4         # t-blocks of 512 over T
NCHUNK = 8      # c_in chunks of 128 over C
NTT = 16        # t-tiles of 128 over T
VGW = 65        # V group width per head (64 cols + ones col)


def _build_nc():
    import concourse.bass as bass
    import concourse.tile as tile
    from concourse import bacc, mybir

    F32 = mybir.dt.float32
    BF16 = mybir.dt.bfloat16
    EXP = mybir.ActivationFunctionType.Exp
    CPY = mybir.ActivationFunctionType.Copy
    IDN = mybir.ActivationFunctionType.Identity

    nc = bacc.Bacc("TRN2", target_bir_lowering=False, debug=False,
                   num_devices=NCORES)

    d_xT = nc.dram_tensor("xT", (C, T), BF16, kind="ExternalInput").ap()
    d_wqk = nc.dram_tensor("wqk", (C, 1024), BF16, kind="ExternalInput").ap()
    d_bqk = nc.dram_tensor("bqk", (128, 8), F32, kind="ExternalInput").ap()
    d_wv = nc.dram_tensor("wv", (C, 512), BF16, kind="ExternalInput").ap()
    d_w2 = nc.dram_tensor("w2", (512, C), BF16, kind="ExternalInput").ap()
    d_mask = nc.dram_tensor("mask", (128, 256), BF16,
                            kind="ExternalInput").ap()
    d_ones = nc.dram_tensor("onesr", (128, 8), BF16,
                            kind="ExternalInput").ap()
    d_out1 = nc.dram_tensor("out1", (T, C), BF16, kind="ExternalOutput").ap()
    d_out2 = nc.dram_tensor("out2", (T, C), BF16, kind="ExternalOutput").ap()

    with tile.TileContext(nc) as tc:
        with tc.tile_pool(name="persist", bufs=1) as persist:
            xts = [[persist.tile([128, 512], BF16, name=f"xt{c}_{tb}",
                                 tag=f"xt{c}_{tb}") for tb in range(NTB)]
                   for c in range(NCHUNK)]
            wv_sb = [persist.tile([128, 512], BF16, name=f"wv{c}",
                                  tag=f"wv{c}") for c in range(NCHUNK)]
            wqkA = [persist.tile([128, 256], BF16, name=f"wqa{c}",
                                 tag=f"wqa{c}") for c in range(NCHUNK)]
            wqkB = [persist.tile([128, 768], BF16, name=f"wqb{c}",
                                 tag=f"wqb{c}") for c in range(NCHUNK)]
            w2_sb = [persist.tile([128, C], BF16, name=f"w2{g}", tag=f"w2{g}")
                     for g in range(4)]
            bq_sb = persist.tile([128, 8], F32, name="bq", tag="bq")
            mk = persist.tile([128, 256], BF16, name="mk", tag="mk")
            vt = [persist.tile([128, HL * VGW], BF16, name=f"v{t}",
                               tag=f"v{t}") for t in range(NTT)]
            qt = [[persist.tile([128, 512], BF16, name=f"qt{g}_{tb}",
                                tag=f"qt{g}_{tb}") for tb in range(NTB)]
                  for g in range(4)]
            kt_ = [[persist.tile([128, 512], BF16, name=f"kt{g}_{tb}",
                                 tag=f"kt{g}_{tb}") for tb in range(NTB)]
                   for g in range(4)]
            ynt = [[persist.tile([128, 512], BF16, name=f"yn{g}_{qb}",
                                 tag=f"yn{g}_{qb}") for qb in range(NTB)]
                   for g in range(4)]

            # ---- input DMAs, ordered so attention can start ~15us in ----
            nc.sync.dma_start(bq_sb[:], d_bqk[:])
            nc.sync.dma_start(mk[:], d_mask[:])
            for t in range(NTT):
                vg0 = vt[t][:].rearrange("p (g c) -> p g c", g=HL)
                nc.sync.dma_start(
                    vg0[:, :, 64:65],
                    d_ones[:, 0:HL].rearrange("p (g o) -> p g o", o=1))
            # wv + xT t-block 0, chunk-interleaved
            for c in range(NCHUNK):
                nc.sync.dma_start(wv_sb[c][:], d_wv[c * 128:(c + 1) * 128, :])
                nc.sync.dma_start(xts[c][0][:],
                                  d_xT[c * 128:(c + 1) * 128, 0:512])
            # wqk part A: the pair-0 quads (q cols 0:128, k cols 512:640)
            for c in range(NCHUNK):
                src = d_wqk[c * 128:(c + 1) * 128, :].rearrange(
                    "p (s w) -> p s w", s=2)
                dst = wqkA[c][:].rearrange("p (s w) -> p s w", s=2)
                nc.sync.dma_start(dst, src[:, :, 0:128])
            # xT t-blocks 1..3
            for tb in range(1, NTB):
                for c in range(NCHUNK):
                    nc.sync.dma_start(
                        xts[c][tb][:],
                        d_xT[c * 128:(c + 1) * 128, tb * 512:(tb + 1) * 512])
            # wqk part B: pairs 1-3 (q cols 128:512, k cols 640:1024)
            for c in range(NCHUNK):
                src = d_wqk[c * 128:(c + 1) * 128, :].rearrange(
                    "p (s w) -> p s w", s=2)
                dst = wqkB[c][:].rearrange("p (s w) -> p s w", s=2)
                nc.sync.dma_start(dst, src[:, :, 128:512])
            for g in range(4):
                nc.sync.dma_start(w2_sb[g][:], d_w2[g * 128:(g + 1) * 128, :])

            def wqk_ap(c, mt):
                if mt == 0:
                    return wqkA[c][:, 0:128]
                if mt == 4:
                    return wqkA[c][:, 128:256]
                if mt < 4:
                    return wqkB[c][:, (mt - 1) * 128:mt * 128]
                return wqkB[c][:, 384 + (mt - 5) * 128:384 + (mt - 4) * 128]

            with tc.tile_pool(name="pt", bufs=6) as pt_pool, \
                 tc.tile_pool(name="yu", bufs=4) as yu_pool, \
                 tc.tile_pool(name="rr", bufs=4) as rr_pool, \
                 tc.tile_pool(name="bc", bufs=4) as bc_pool, \
                 tc.tile_pool(name="rdram", bufs=8, space="DRAM") as rd_pool, \
                 tc.tile_pool(name="outp", bufs=4) as ob_pool, \
                 tc.tile_pool(name="psp", bufs=2, space="PSUM") as ps_p, \
                 tc.tile_pool(name="pss", bufs=2, space="PSUM") as ps_s, \
                 tc.tile_pool(name="psy", bufs=2, space="PSUM") as ps_y:

                # ---- V production (streams behind the xT upload) ----
                with nc.named_scope("vproj"):
                    for tb in range(NTB):
                        for tt in range(4):
                            ps = ps_p.tile([128, 512], F32, name="psp",
                                           tag="psp")
                            for c in range(NCHUNK):
                                nc.tensor.matmul(
                                    ps[:], xts[c][tb][:, tt * 128:(tt + 1) * 128],
                                    wv_sb[c][:],
                                    start=(c == 0), stop=(c == NCHUNK - 1))
                            vg = vt[4 * tb + tt][:].rearrange(
                                "p (g c) -> p g c", g=HL)
                            # ACT is idle during the ramp; evict there
                            nc.scalar.activation(
                                vg[:, :, 0:64],
                                ps[:].rearrange("p (g c) -> p g c", g=HL),
                                CPY)

                def qk_pair(hp):
                    with nc.named_scope(f"qk{hp}"):
                        for tb in range(NTB):
                            for mt in (hp, hp + 4):
                                dst = (qt if mt < 4 else kt_)[hp][tb]
                                ps = ps_p.tile([128, 512], F32, name="psp",
                                               tag="psp")
                                for c in range(NCHUNK):
                                    nc.tensor.matmul(
                                        ps[:], wqk_ap(c, mt), xts[c][tb][:],
                                        start=(c == 0),
                                        stop=(c == NCHUNK - 1))
                                if hp == 0:
                                    nc.scalar.activation(
                                        dst[:], ps[:], IDN,
                                        bias=bq_sb[:, mt:mt + 1])
                                else:
                                    nc.vector.tensor_scalar_add(
                                        dst[:], ps[:], bq_sb[:, mt:mt + 1])

                def attention_pair(hp):
                    with nc.named_scope(f"attn{hp}"):
                        for qb in range(NTB):
                            nkt = 4 * qb + 4
                            psY = {}
                            for side in (0, 1):
                                psY[side] = ps_y.tile([VGW, 512], F32,
                                                      name="psy", tag="psy")
                            for ktile in range(nkt):
                                m = ktile - 4 * qb
                                r0 = 128 * max(m, 0)
                                kb, kc = divmod(ktile, 4)
                                psS = ps_s.tile([128, 1024], F32,
                                                name="pss", tag="pss")
                                pt = pt_pool.tile([128, 1024], BF16,
                                                  name="pt", tag="pt")
                                # S^T: two heads row-tiled, concurrent
                                for side, po in ((0, 0), (1, 64)):
                                    nc.tensor.matmul(
                                        psS[:, side * 512 + r0:(side + 1) * 512],
                                        kt_[hp][kb][po:po + 64,
                                                    kc * 128:(kc + 1) * 128],
                                        qt[hp][qb][po:po + 64, r0:512])
                                if m <= 0:
                                    nc.scalar.activation(pt[:], psS[:], EXP,
                                                         scale=0.125)
                                else:
                                    ps2 = psS[:].rearrange("p (s w) -> p s w",
                                                           s=2)
                                    pt2 = pt[:].rearrange("p (s w) -> p s w",
                                                          s=2)
                                    nc.scalar.activation(pt2[:, :, r0:512],
                                                         ps2[:, :, r0:512],
                                                         EXP, scale=0.125)
                                if m >= 0:
                                    # mask only the 128-wide diagonal band
                                    pt2 = pt[:].rearrange("p (s w) -> p s w",
                                                          s=2)
                                    mkb = mk[:].rearrange("p (s w) -> p s w",
                                                          s=2)
                                    nc.gpsimd.tensor_mul(
                                        pt2[:, :, r0:r0 + 128],
                                        pt2[:, :, r0:r0 + 128], mkb)
                                for side, po in ((0, 0), (1, 64)):
                                    h = 2 * hp + side
                                    nc.tensor.matmul(
                                        psY[side][:, r0:512],
                                        vt[ktile][:, h * VGW:(h + 1) * VGW],
                                        pt[:, side * 512 + r0:(side + 1) * 512],
                                        start=(ktile == 0),
                                        stop=(ktile == nkt - 1))
                            # evict unnormalized; normalize via DMA-chain
                            # ([1,512] -> [128,4] reshape round trip so the
                            # reciprocal runs wide on DVE)
                            for side, po in ((0, 0), (1, 64)):
                                yu = yu_pool.tile([VGW, 512], F32, name="yu",
                                                  tag="yu")
                                nc.vector.tensor_copy(yu[:], psY[side][:])
                                rd = rd_pool.tile([1, 512], F32, name="rd",
                                                  tag="rd")
                                nc.sync.dma_start(rd[:], yu[64:65, :])
                                rsq = rr_pool.tile([128, 4], F32, name="rsq",
                                                   tag="rsq")
                                nc.sync.dma_start(
                                    rsq[:],
                                    rd[:].rearrange("o (p f) -> (o p) f",
                                                    p=128))
                                rqr = rr_pool.tile([128, 4], F32, name="rqr",
                                                   tag="rqr")
                                nc.vector.reciprocal(rqr[:], rsq[:])
                                rdr = rd_pool.tile([1, 512], F32, name="rdr",
                                                   tag="rdr")
                                nc.sync.dma_start(
                                    rdr[:].rearrange("o (p f) -> (o p) f",
                                                     p=128),
                                    rqr[:])
                                bc = bc_pool.tile([64, 512], F32, name="bc",
                                                  tag="bc")
                                rd_ap = rdr[:]
                                nc.sync.dma_start(
                                    bc[:],
                                    bass.AP(tensor=rd_ap.tensor,
                                            offset=rd_ap.offset,
                                            ap=[[0, 64]] + list(rd_ap.ap[1:])))
                                nc.vector.tensor_mul(
                                    ynt[hp][qb][po:po + 64, :],
                                    yu[0:64, :], bc[:])

                def cproj_half(gs, d_out, on_scalar):
                    with nc.named_scope(f"cproj{gs[0]}"):
                        for qb in range(NTB):
                            for tt in range(4):
                                t = 4 * qb + tt
                                for nb in range(2):
                                    nsl = slice(nb * 512, (nb + 1) * 512)
                                    ps = ps_p.tile([128, 512], F32, name="psp",
                                                   tag="psp")
                                    for j, g in enumerate(gs):
                                        nc.tensor.matmul(
                                            ps[:],
                                            ynt[g][qb][:, tt * 128:(tt + 1) * 128],
                                            w2_sb[g][:, nsl],
                                            start=(j == 0),
                                            stop=(j == len(gs) - 1))
                                    ob = ob_pool.tile([128, 512], BF16,
                                                      name="ob", tag="ob")
                                    if on_scalar:
                                        nc.scalar.activation(ob[:], ps[:],
                                                             CPY)
                                    else:
                                        nc.vector.tensor_copy(ob[:], ps[:])
                                    nc.sync.dma_start(
                                        d_out[t * 128:(t + 1) * 128, nsl],
                                        ob[:])

                # interleave: QK(hp+1) overlaps attention(hp); cproj of
                # heads 0-3 fills PE gaps during attention of pair 3;
                # cproj of heads 4-7 streams per q-block behind attn3.
                qk_pair(0)
                for hp in range(4):
                    if hp + 1 < 4:
                        qk_pair(hp + 1)
                    if hp == 3:
                        cproj_half((0, 1), d_out1, False)
                    attention_pair(hp)
                cproj_half((2, 3), d_out2, True)

    nc.compile()
    return nc


def _shard_inputs(x, c_attn_w, c_attn_b, c_proj_w):
    import ml_dtypes
    bf16 = ml_dtypes.bfloat16
    i = np.arange(128)[:, None]
    j = np.arange(128)[None, :]
    tri = (i <= j)
    mask = np.concatenate([tri, tri], axis=1).astype(bf16)
    ones8 = np.ones((128, 8), bf16)
    in_maps = []
    for core in range(NCORES):
        b, half = core // 2, core % 2
        h0 = half * HL
        lo, hi = h0 * D, (h0 + HL) * D
        wq = c_attn_w[:, lo:hi]
        wk = c_attn_w[:, C + lo:C + hi]
        wv = c_attn_w[:, 2 * C + lo:2 * C + hi]
        bqk = np.concatenate([c_attn_b[lo:hi], c_attn_b[C + lo:C + hi]])
        in_maps.append({
            "xT": np.ascontiguousarray(x[b].T).astype(bf16),
            "wqk": np.ascontiguousarray(
                np.concatenate([wq, wk], axis=1)).astype(bf16),
            "bqk": np.ascontiguousarray(
                bqk.reshape(8, 128).T, dtype=np.float32),
            "wv": np.ascontiguousarray(wv).astype(bf16),
            "w2": np.ascontiguousarray(c_proj_w[lo:hi, :]).astype(bf16),
            "mask": mask,
            "onesr": ones8,
        })
    return in_maps


def _run(x, c_attn_w, c_attn_b, c_proj_w, c_proj_b, trace=False):
    from concourse import bass_utils
    x = np.asarray(x, np.float32)
    c_attn_w = np.asarray(c_attn_w, np.float32)
    c_attn_b = np.asarray(c_attn_b, np.float32)
    c_proj_w = np.asarray(c_proj_w, np.float32)
    c_proj_b = np.asarray(c_proj_b, np.float32)

    nc = _build_nc()
    in_maps = _shard_inputs(x, c_attn_w, c_attn_b, c_proj_w)
    res = bass_utils.run_bass_kernel_spmd(nc, in_maps,
                                          core_ids=list(range(NCORES)),
                                          trace=trace)
    # softmax rows sum to 1 => the V bias contributes bv^T @ c_proj_w to
    # every (b, t) row; fold it into the host-side bias add.
    bias = c_proj_b + c_attn_b[2 * C:] @ c_proj_w
    y = np.empty((B, T, C), np.float32)
    for b in range(B):
        acc = None
        for core in (2 * b, 2 * b + 1):
            for name in ("out1", "out2"):
                part = np.asarray(res.results[core][name], np.float32)
                acc = part if acc is None else acc + part
        y[b] = acc + bias[None, :]
    return y, res


def kernel(x, c_attn_w, c_attn_b, c_proj_w, c_proj_b):
    y, _ = _run(x, c_attn_w, c_attn_b, c_proj_w, c_proj_b, trace=False)
    return y
